# revision 35
# baseline (speedup 1.0000x reference)
"""PairEmbedding Bass kernel for 8 TRN2 NeuronCores.

out[b,i,j,:] = Co[b,j,:] + Cp[b,i,:] + sep(b,i,j) * w_sep
  Co[j] = se_j @ W1 + [0 | pe_j]
  Cp[i] = se_i @ W2 + b_proj + [pe_i | 0]
  sep(i,j) = ln(|aa_i - aa_j| + 1)
where se = emb_table[seq], pe = pos_table[aa_idx], W1 = W_proj[0:144],
W2 = W_proj[144:288], w_sep = W_proj[288].

Sharding: core c -> batch b = c//2, row block i in [128*(c%2), 128*(c%2)+128),
all 256 j. Per-core output (128, 256*288) f16 (upcast to f32 on host).

Steady state, per 4096-f32-column cycle (18 cycles cover the 73728-column
flat (j,d) space):
  PE: ~9 matmuls of N<=512 (PSUM bank-aligned pieces, split at 2304-column
      group boundaries). lhsT = Gall group slice (K=9: [1; sep rows for the
      group's 8 j's]); rhs = R_ALL columns ([co f16; 8 wsep pattern rows]).
      Gives Co[j] + sep[:,j]*wsep in f32 PSUM. Pattern rows are zero
      outside their j's columns, so any piece inside a group works with
      the group's lhsT.
  ACT: converts PSUM->f16 for the two 1536-wide units of the cycle.
  DVE: adds cp (phase-shifted periodic replica) to those, and does a fused
      convert+add for the 1024-wide unit straight from PSUM.
  SP: one DMA of the assembled [128, 4096] f16 tile per cycle.
Engine budget per pass: PE ~58us (35us streaming + ~140ns/matmul weight
load), DMA ~53us (f16 output at ~360 GB/s), ACT ~53us, DVE ~54us.
GPSIMD is unused: it cannot read PSUM and its adds measure far slower
than modeled.
"""

import math
from contextlib import ExitStack

import numpy as np

from concourse import bacc, bass, mybir, tile
from concourse.bass_utils import run_bass_kernel_spmd

dt = mybir.dt
AF = mybir.ActivationFunctionType
ALU = mybir.AluOpType

B = 4
L = 256
D_PAIR = 288
D_HALF = 144
MAX_LEN = 260
VOCAB = 21
IH = 128            # i rows per core
JG = 8              # j's per lhsT group (pattern period 8*288 = 2304)
NGRP = L // JG      # 32 lhsT groups
KR = 1 + JG         # lhsT rows: [co; 8 sep rows] (single-f16 Co is plenty
                    # accurate for the 2e-2 rel-err budget)
GRPW = JG * D_PAIR  # 2304 flat columns per group
FW = L * D_PAIR     # 73728 flat columns
CYC = 4096          # PSUM f32 columns per cycle (8 banks)
NCYC = FW // CYC    # 18 cycles per pass
N_CORES = 8

# per-cycle engine schedule for the 3 units: A = ACT convert + DVE f16 add,
# D = DVE fused convert+add from PSUM. GPSIMD can't read PSUM and its adds
# measure far slower than modeled, so it is unused.
SCHED3 = "AAD"


def _pos_enc_table() -> np.ndarray:
    idx = np.arange(0, D_HALF, 2, dtype=np.float32)
    t = (np.float32(math.log(10000.0)) * idx) / np.float32(D_HALF)
    denom = np.exp(t, dtype=np.float32)
    pos = np.arange(MAX_LEN, dtype=np.float32)[:, None]
    pe = np.zeros((MAX_LEN, D_HALF), dtype=np.float32)
    pe[:, 0::2] = np.sin(pos / denom, dtype=np.float32)
    pe[:, 1::2] = np.cos(pos / denom, dtype=np.float32)
    return pe


def _bcast(ap_src, nparts: int):
    return bass.AP(
        tensor=ap_src.tensor, offset=ap_src.offset, ap=[[0, nparts], *ap_src.ap]
    )


def build(stage: str = "full", repeat: int = 1, variant: str = "") -> bass.Bass:
    nc = bacc.Bacc("TRN2", target_bir_lowering=False)

    seqb_d = nc.dram_tensor("seqb", [L], dt.int32, kind="ExternalInput")
    seqi_d = nc.dram_tensor("seqi", [IH], dt.int32, kind="ExternalInput")
    aab_d = nc.dram_tensor("aab", [L], dt.int32, kind="ExternalInput")
    aai_d = nc.dram_tensor("aai", [IH], dt.int32, kind="ExternalInput")
    emb_d = nc.dram_tensor("emb", [VOCAB, D_HALF], dt.float32, kind="ExternalInput")
    wp_d = nc.dram_tensor("wp", [D_PAIR + 1, D_PAIR], dt.float32, kind="ExternalInput")
    bp_d = nc.dram_tensor("bp", [D_PAIR], dt.float32, kind="ExternalInput")
    out_d = nc.dram_tensor("out", [IH, FW], dt.float16, kind="ExternalOutput")

    # pos-table gather sources, pre-arranged on host: chunk c of <=128 pos
    # rows on partitions, channel slice [0:144] (posL, pe_i) or [144:288]
    # (posR, pe_j), zero elsewhere.
    pos_np = _pos_enc_table()
    posL_np = np.zeros((128, 3 * D_PAIR), dtype=np.float32)
    posR_np = np.zeros((128, 3 * D_PAIR), dtype=np.float32)
    for c in range(3):
        rows = 128 if c < 2 else MAX_LEN - 256
        chunk = pos_np[c * 128 : c * 128 + rows, :]
        posL_np[0:rows, c * D_PAIR : c * D_PAIR + D_HALF] = chunk
        posR_np[0:rows, c * D_PAIR + D_HALF : (c + 1) * D_PAIR] = chunk
    posL_d = nc.inline_tensor(posL_np, "posL_c")
    posR_d = nc.inline_tensor(posR_np, "posR_c")
    iota_np = (
        np.arange(128, dtype=np.float32)[:, None]
        + 128.0 * np.arange(3, dtype=np.float32)[None, :]
    ).astype(np.float32)
    iota_d = nc.inline_tensor(iota_np, "iota")
    ones1_d = nc.inline_tensor(np.ones((1, NGRP * IH), dtype=np.float16), "ones1")

    sched = SCHED3
    for v in variant.split("+"):
        if v.startswith("sched"):
            sched = v[5:]
    assert len(sched) == 3
    # conv/add units per cycle: the D (DVE-fused) unit is 1024 wide, A units
    # 1536; unit order follows the sched string
    units = []
    lo = 0
    for ch in sched:
        w = 1024 if ch == "D" else 1536
        units.append((lo, lo + w))
        lo += w
    assert lo == CYC, sched

    with tile.TileContext(nc) as tc, ExitStack() as ctx:
        persist = ctx.enter_context(tc.tile_pool(name="persist", bufs=1))
        psp = ctx.enter_context(tc.tile_pool(name="psp", bufs=1, space="PSUM"))

        # R_ALL rows: 0=co (f16), 1..8 = wsep at j%8==jj
        rall_t = persist.tile([KR, FW], dt.float16, tag="rall")
        # Gall rows: 0=ones, 1..8 = sep[i, 8g+jj]; free = g*128 + i
        gall_t = persist.tile([KR, NGRP * IH], dt.float16, tag="gall")
        # periodic cp replica: cp_rep[:, c] = cp[:, c % 288], c < 288+1536
        CPW = D_PAIR + 1536
        cp_rep = persist.tile([IH, CPW], dt.float16, tag="cprep")
        ps_all = psp.tile([128, CYC], dt.float32, tag="psall")

        with ExitStack() as pre:
            scr = pre.enter_context(tc.tile_pool(name="scr", bufs=1))

            # ---- input loads ----
            iota_t = scr.tile([128, 3], dt.float32, tag="iota")
            nc.sync.dma_start(iota_t, iota_d[:, :])

            emb_t = scr.tile([VOCAB, D_HALF], dt.float32, tag="emb")
            nc.sync.dma_start(emb_t, emb_d[:, :])

            w1a = scr.tile([128, D_PAIR], dt.float32, tag="w1a")
            nc.sync.dma_start(w1a, wp_d[0:128, :])
            w1b = scr.tile([16, D_PAIR], dt.float32, tag="w1b")
            nc.sync.dma_start(w1b, wp_d[128:144, :])
            w2a = scr.tile([128, D_PAIR], dt.float32, tag="w2a")
            nc.sync.dma_start(w2a, wp_d[144:272, :])
            w2b = scr.tile([16, D_PAIR], dt.float32, tag="w2b")
            nc.sync.dma_start(w2b, wp_d[272:288, :])
            wsep_f = scr.tile([1, D_PAIR], dt.float32, tag="wsepf")
            nc.sync.dma_start(wsep_f, wp_d[288:289, :])

            bp_t = scr.tile([1, D_PAIR], dt.float32, tag="bp")
            nc.sync.dma_start(bp_t, bp_d[:])

            seqB_i = scr.tile([VOCAB, L], dt.int32, tag="seqBi")
            nc.sync.dma_start(seqB_i, _bcast(seqb_d[:], VOCAB))
            seqI_i = scr.tile([VOCAB, IH], dt.int32, tag="seqIi")
            nc.sync.dma_start(seqI_i, _bcast(seqi_d[:], VOCAB))
            aaIB_i = scr.tile([128, IH], dt.int32, tag="aaIBi")
            nc.sync.dma_start(aaIB_i, _bcast(aai_d[:], 128))
            aaB_i = scr.tile([128, L], dt.int32, tag="aaBi")
            nc.sync.dma_start(aaB_i, _bcast(aab_d[:], 128))
            # aaB8[jj, g] = aa[8g+jj]
            aaB8_i = scr.tile([JG, NGRP], dt.int32, tag="aaB8i")
            nc.sync.dma_start(
                aaB8_i,
                bass.AP(tensor=aab_d[:].tensor, offset=0, ap=[[1, JG], [JG, NGRP]]),
            )

            posL = scr.tile([128, 3 * D_PAIR], dt.float32, tag="posL")
            nc.sync.dma_start(posL, posL_d[:, :])
            posR = scr.tile([128, 3 * D_PAIR], dt.float32, tag="posR")
            nc.sync.dma_start(posR, posR_d[:, :])

            nc.sync.dma_start(gall_t[0:1, :], ones1_d[:, :])

            # ---- int -> f32 casts ----
            seqB_f = scr.tile([VOCAB, L], dt.float32, tag="seqBf")
            nc.vector.tensor_copy(seqB_f, seqB_i)
            seqI_f = scr.tile([VOCAB, IH], dt.float32, tag="seqIf")
            nc.vector.tensor_copy(seqI_f, seqI_i)
            aaIB_f = scr.tile([128, IH], dt.float32, tag="aaIBf")
            nc.vector.tensor_copy(aaIB_f, aaIB_i)
            aaB_f = scr.tile([128, L], dt.float32, tag="aaBf")
            nc.vector.tensor_copy(aaB_f, aaB_i)
            aaB8_f = scr.tile([JG, NGRP], dt.float32, tag="aaB8f")
            nc.vector.tensor_copy(aaB8_f, aaB8_i)

            # ---- one-hots ----
            ohSeq = scr.tile([VOCAB, L], dt.float32, tag="ohSeq")
            nc.vector.tensor_scalar(
                ohSeq, seqB_f, iota_t[0:VOCAB, 0:1], None, ALU.is_equal
            )
            ohSeqI = scr.tile([VOCAB, IH], dt.float32, tag="ohSeqI")
            nc.vector.tensor_scalar(
                ohSeqI, seqI_f, iota_t[0:VOCAB, 0:1], None, ALU.is_equal
            )
            ohP = []
            ohPi = []
            for c in range(3):
                t = scr.tile([128, L], dt.float32, tag=f"ohP{c}", name=f"ohP{c}")
                nc.vector.tensor_scalar(t, aaB_f, iota_t[:, c : c + 1], None, ALU.is_equal)
                ohP.append(t)
                ti = scr.tile([128, IH], dt.float32, tag=f"ohPi{c}", name=f"ohPi{c}")
                nc.vector.tensor_scalar(
                    ti, aaIB_f, iota_t[:, c : c + 1], None, ALU.is_equal
                )
                ohPi.append(ti)

            # ---- seT = emb^T gathered by seq: (144, L) split 128+16 rows ----
            seT_a_ps = ps_all[:, 0:L]
            nc.tensor.matmul(seT_a_ps, emb_t[:, 0:128], ohSeq, start=True, stop=True)
            seT_b_ps = ps_all[0:16, 256 : 256 + L]
            nc.tensor.matmul(
                seT_b_ps, emb_t[:, 128:D_HALF], ohSeq, start=True, stop=True
            )
            seT_a = scr.tile([128, L], dt.float32, tag="seTa")
            nc.vector.tensor_copy(seT_a, seT_a_ps)
            seT_b = scr.tile([16, L], dt.float32, tag="seTb")
            nc.vector.tensor_copy(seT_b, seT_b_ps)

            seTi_a_ps = ps_all[:, 512 : 512 + IH]
            nc.tensor.matmul(seTi_a_ps, emb_t[:, 0:128], ohSeqI, start=True, stop=True)
            seTi_b_ps = ps_all[0:16, 640 : 640 + IH]
            nc.tensor.matmul(
                seTi_b_ps, emb_t[:, 128:D_HALF], ohSeqI, start=True, stop=True
            )
            seTi_a = scr.tile([128, IH], dt.float32, tag="seTia")
            nc.vector.tensor_copy(seTi_a, seTi_a_ps)
            seTi_b = scr.tile([16, IH], dt.float32, tag="seTib")
            nc.vector.tensor_copy(seTi_b, seTi_b_ps)

            # ---- Co halves -> f16 hi/lo -> R_ALL rows 0/1 ----
            for h in range(2):
                co_ps = ps_all[:, 1024 + 512 * h : 1024 + 512 * h + D_PAIR]
                sl = slice(h * 128, (h + 1) * 128)
                nc.tensor.matmul(co_ps, seT_a[:, sl], w1a, start=True, stop=False)
                nc.tensor.matmul(co_ps, seT_b[:, sl], w1b, start=False, stop=False)
                for c in range(3):
                    nc.tensor.matmul(
                        co_ps,
                        ohP[c][:, sl],
                        posR[:, c * D_PAIR : (c + 1) * D_PAIR],
                        start=False,
                        stop=(c == 2),
                    )
                co_hi = scr.tile(
                    [128, D_PAIR], dt.float16, tag=f"cohi{h}", name=f"cohi{h}"
                )
                nc.vector.tensor_copy(co_hi, co_ps)
                dst = slice(h * 128 * D_PAIR, (h * 128 + 128) * D_PAIR)
                nc.sync.dma_start(rall_t[0:1, dst], co_hi)

            # ---- Cp -> periodic f16 replica cp_rep ----
            ones_f = scr.tile([1, IH], dt.float32, tag="onesf")
            nc.vector.memset(ones_f, 1.0)
            cp_ps = ps_all[:, 2048 : 2048 + D_PAIR]
            nc.tensor.matmul(cp_ps, seTi_a, w2a, start=True, stop=False)
            nc.tensor.matmul(cp_ps, seTi_b, w2b, start=False, stop=False)
            for c in range(3):
                nc.tensor.matmul(
                    cp_ps,
                    ohPi[c],
                    posL[:, c * D_PAIR : (c + 1) * D_PAIR],
                    start=False,
                    stop=False,
                )
            nc.tensor.matmul(cp_ps, ones_f, bp_t, start=False, stop=True)
            nc.vector.tensor_copy(cp_rep[:, 0:D_PAIR], cp_ps)
            x = D_PAIR
            while x < CPW:
                step = min(x, CPW - x)
                nc.vector.tensor_copy(cp_rep[:, x : x + step], cp_rep[:, 0:step])
                x += step

            # ---- wsep pattern rows 1..8 of R_ALL ----
            # zero-fill + wsep seeds via DMA (compute engines can't start at
            # partition 1), then period-doubling DMAs
            wsep16 = scr.tile([1, D_PAIR], dt.float16, tag="wsep16")
            nc.vector.tensor_copy(wsep16, wsep_f)
            zt = scr.tile([JG, 1152], dt.float16, tag="zt")
            nc.vector.memset(zt, 0.0)
            for q in range(GRPW // 1152):
                nc.sync.dma_start(rall_t[1 : 1 + JG, q * 1152 : (q + 1) * 1152], zt)
            for jj in range(JG):
                nc.sync.dma_start(
                    rall_t[1 + jj : 2 + jj, jj * D_PAIR : (jj + 1) * D_PAIR], wsep16
                )
            # doubling; steps stay multiples of the 2304 period and under the
            # 64KB SDMA descriptor cap (<= 32256 f16 = 64512B)
            x = GRPW
            while x < FW:
                step = min(x, FW - x, 14 * GRPW)
                nc.sync.dma_start(
                    rall_t[1 : 1 + JG, x : x + step], rall_t[1 : 1 + JG, 0:step]
                )
                x += step

            # ---- sep rows of Gall: row 1+jj, col g*128+i = ln(|aa_{8g+jj}-aa_i|+1) ----
            s16_all = scr.tile([JG, NGRP * IH], dt.float16, tag="s16all")
            HG = NGRP // 2
            for hh in range(2):
                d_half = scr.tile([JG, HG * IH], dt.float32, tag="dhalf", name="dhalf")
                for gg in range(HG):
                    g = hh * HG + gg
                    nc.vector.tensor_scalar(
                        d_half[:, gg * IH : (gg + 1) * IH],
                        aaIB_f[0:JG, :],
                        aaB8_f[:, g : g + 1],
                        None,
                        ALU.subtract,
                    )
                nc.scalar.activation(d_half, d_half, AF.Abs)
                nc.scalar.activation(d_half, d_half, AF.Ln, bias=1.0)
                nc.vector.tensor_copy(
                    s16_all[:, hh * HG * IH : (hh + 1) * HG * IH], d_half
                )
            nc.sync.dma_start(gall_t[1 : 1 + JG, :], s16_all)

        if stage == "setup":
            dbg = ctx.enter_context(tc.tile_pool(name="dbg", bufs=1))
            dbf = dbg.tile([IH, CPW], dt.float16, tag="dbf")
            nc.vector.tensor_copy(dbf, cp_rep)
            nc.sync.dma_start(out_d[:, 0:CPW], dbf)
            return nc

        # ---- steady loop ----
        obp = ctx.enter_context(tc.tile_pool(name="obp", bufs=4))
        cobp = ctx.enter_context(tc.tile_pool(name="cobp", bufs=5))
        ncyc = int(stage[5:]) if stage.startswith("jloop") else NCYC
        noconv = "noconv" in variant
        nodma = "nodma" in variant
        if variant == "dmaonly":
            obs = []
            for k in range(2):
                t = obp.tile([IH, CYC], dt.float16, tag="ob", name="ob")
                nc.vector.memset(t, 0.5)
                obs.append(t)
            for k in range(ncyc * repeat):
                k = k % ncyc
                nc.sync.dma_start(out_d[:, k * CYC : (k + 1) * CYC], obs[k % 2])
            return nc

        obs = []
        if noconv:
            for k in range(2):
                t = obp.tile([IH, CYC], dt.float16, tag="ob", name="ob")
                nc.vector.memset(t, 0.5)
                obs.append(t)
        # matmul pieces per cycle: 512-bank chunks, split where a 2304-column
        # group boundary falls inside a chunk (the lhsT sep rows only match
        # one group's j's)
        cycle_pieces = []
        for k in range(NCYC):
            pieces = []
            for c in range(8):
                lo = k * CYC + 512 * c
                hi = lo + 512
                b = (lo // GRPW + 1) * GRPW
                if b < hi:
                    pieces.append((lo, b))
                    lo = b
                pieces.append((lo, hi))
            cycle_pieces.append(pieces)

        for ki in range(ncyc * repeat):
            k = ki % ncyc
            base = k * CYC  # flat column base of this cycle
            for lo, hi in cycle_pieces[k]:
                g = lo // GRPW
                nc.tensor.matmul(
                    ps_all[:, lo - base : hi - base],
                    gall_t[:, g * IH : (g + 1) * IH],
                    rall_t[:, lo:hi],
                    start=True,
                    stop=True,
                )
            if noconv:
                if not nodma:
                    nc.sync.dma_start(
                        out_d[:, base : base + CYC], obs[ki % 2]
                    )
                continue
            ob = obp.tile([IH, CYC], dt.float16, tag="ob", name="ob")
            for u in range(len(units)):
                lo, hi = units[u]
                w = hi - lo
                phase = (base + lo) % D_PAIR
                cps = cp_rep[:, phase : phase + w]
                if sched[u] == "D":
                    nc.vector.tensor_tensor(
                        ob[:, lo:hi], ps_all[:, lo:hi], cps, ALU.add
                    )
                else:
                    cob = cobp.tile([IH, 1536], dt.float16, tag="cob", name="cob")
                    nc.scalar.copy(cob[:, 0:w], ps_all[:, lo:hi])
                    nc.vector.tensor_tensor(ob[:, lo:hi], cob[:, 0:w], cps, ALU.add)
            if not nodma:
                nc.sync.dma_start(out_d[:, base : base + CYC], ob)

    return nc


_NC_CACHE = []


def make_in_maps(seq, aa_idx, emb_table, W_proj, b_proj):
    seq = np.asarray(seq, dtype=np.int32)
    aa_idx = np.asarray(aa_idx, dtype=np.int32)
    emb_table = np.ascontiguousarray(np.asarray(emb_table, dtype=np.float32))
    W_proj = np.ascontiguousarray(np.asarray(W_proj, dtype=np.float32))
    b_proj = np.ascontiguousarray(np.asarray(b_proj, dtype=np.float32))
    in_maps = []
    for c in range(N_CORES):
        b, ih = c // 2, c % 2
        in_maps.append(
            {
                "seqb": np.ascontiguousarray(seq[b]),
                "seqi": np.ascontiguousarray(seq[b, ih * IH : (ih + 1) * IH]),
                "aab": np.ascontiguousarray(aa_idx[b]),
                "aai": np.ascontiguousarray(aa_idx[b, ih * IH : (ih + 1) * IH]),
                "emb": emb_table,
                "wp": W_proj,
                "bp": b_proj,
            }
        )
    return in_maps


def gather_out(results) -> np.ndarray:
    out = np.empty((B, L, L, D_PAIR), dtype=np.float32)
    for c in range(N_CORES):
        b, ih = c // 2, c % 2
        out[b, ih * IH : (ih + 1) * IH] = (
            np.asarray(results[c]["out"]).astype(np.float32).reshape(IH, L, D_PAIR)
        )
    return out


def kernel(seq, aa_idx, emb_table, W_proj, b_proj) -> np.ndarray:
    if not _NC_CACHE:
        nc = build()
        nc.finalize()
        _NC_CACHE.append(nc)
    nc = _NC_CACHE[0]
    in_maps = make_in_maps(seq, aa_idx, emb_table, W_proj, b_proj)
    res = run_bass_kernel_spmd(nc, in_maps, core_ids=list(range(N_CORES)))
    return gather_out(res.results)
